# revision 15
# baseline (speedup 1.0000x reference)
"""Bilateral filter (5x5, sigma_space = sigma_density = 1.1) on 8 TRN2 NeuronCores.

Contract: kernel(x, gw) takes FULL inputs
    x : [4, 3, 512, 512] float32
    gw: [5, 5] float32 (normalized spatial gaussian)
returns FULL output [4, 3, 512, 512] float32.

Sharding: pure data parallel over H. Core k processes output rows
[64k, 64k+64) of every (b, c) channel. The host pre-pads x with 2-pixel
edge replication on H and W and hands each core a [12, 68, 516] strip
(12 = 4*3 channels, 68 = 64 rows + 2+2 halo, 516 = 512 + 2+2 pad), so the
device kernel needs no boundary handling and no inter-core communication.

Device algorithm (v1, direct, fp32):
  out = sum_t(w_t * p_t) / sum_t(w_t),  w_t = gw_t * exp(-(p_t - c)^2 / (2 s^2))
(the reference's per-window normalization of the density weights cancels in
the num/den ratio, so it is skipped).
Per 128-row tile: 5 row-shifted copies of the strip are DMA'd into SBUF so
every tap is a free-dim slice; per tap: DVE subtract, ACT Square, ACT
Exp(-u + ln(gw_t)) (folds the spatial weight in as the activation bias),
DVE multiply + two DVE accumulate-adds.
"""

import math

import numpy as np

import concourse.bass as bass
import concourse.bacc as bacc
import concourse.tile as tile
from concourse import mybir
from concourse.bass_utils import run_bass_kernel_spmd

# ---- problem constants (hardcoded per contract) ----
B, C, H, W = 4, 3, 512, 512
K = 5
PAD = 2
SIGMA = 0.3 * ((K - 1) * 0.5 - 1) + 0.8  # 1.1
NCORES = 8
CH = B * C                # 12 channels
ROWS_PER_CORE = H // NCORES   # 64
SH = ROWS_PER_CORE + 2 * PAD  # 68 rows per channel strip (with halo)
SW = W + 2 * PAD              # 516
P = 128                   # SBUF partitions
TILES = CH * ROWS_PER_CORE // P  # 6 tiles of 128 output rows per core
CH_PER_TILE = P // ROWS_PER_CORE  # 2 channels per tile

FP32 = mybir.dt.float32


def _build_nc(gw: np.ndarray) -> bass.Bass:
    inv2s2 = 1.0 / (2.0 * SIGMA * SIGMA)
    sq_scale = math.sqrt(inv2s2)  # Square(d * sq_scale) = d^2/(2 s^2)
    lgw = np.log(np.asarray(gw, np.float64)).astype(np.float32)  # [5,5]

    nc = bacc.Bacc(None)
    xp = nc.declare_dram_parameter("xp", [CH, SH, SW], FP32, isOutput=False)
    out = nc.declare_dram_parameter("out", [CH, ROWS_PER_CORE, W], FP32, isOutput=True)

    with tile.TileContext(nc) as tc:
        with (
            tc.tile_pool(name="singles", bufs=1) as singles,
            tc.tile_pool(name="shift", bufs=2) as shift_pool,
            tc.tile_pool(name="acc", bufs=4) as acc_pool,
            tc.tile_pool(name="tmp", bufs=4) as tmp_pool,
            tc.tile_pool(name="res", bufs=2) as res_pool,
        ):
            # per-tap activation bias ln(gw[i,j]) as [P,1] columns
            lgw_tile = singles.tile([P, K * K], FP32, tag="lgw")
            for idx in range(K * K):
                nc.vector.memset(
                    lgw_tile[:, idx:idx + 1], float(lgw[idx // K, idx % K])
                )
            for t in range(TILES):
                c0 = t * CH_PER_TILE
                # 5 row-shifted copies: S[p, i, :] = xp[ch(p), row(p)+i, :]
                # (plain 2D DMAs only — partition-split views mis-lower)
                S = shift_pool.tile([P, K, SW], FP32, tag="shift")
                for i in range(K):
                    for c in range(CH_PER_TILE):
                        nc.sync.dma_start(
                            out=S[c * ROWS_PER_CORE:(c + 1) * ROWS_PER_CORE,
                                  i, :],
                            in_=xp[c0 + c, i:i + ROWS_PER_CORE, :],
                        )
                center = S[:, PAD, PAD:PAD + W]

                num = acc_pool.tile([P, W], FP32, tag="num")
                den = acc_pool.tile([P, W], FP32, tag="den")

                first = True
                for i in range(K):
                    for j in range(K):
                        patch = S[:, i, j:j + W]
                        d = tmp_pool.tile([P, W], FP32, tag="d")
                        nc.vector.tensor_sub(d[:, :], patch, center)
                        # u = d^2 / (2 s^2)
                        nc.scalar.activation(
                            d[:, :], d[:, :],
                            mybir.ActivationFunctionType.Square,
                            scale=sq_scale,
                        )
                        # e = gw[i,j] * exp(-u)  (spatial weight folded into bias)
                        e_dst = den if first else tmp_pool.tile([P, W], FP32, tag="e")
                        idx = i * K + j
                        nc.scalar.activation(
                            e_dst[:, :], d[:, :],
                            mybir.ActivationFunctionType.Exp,
                            bias=lgw_tile[:, idx:idx + 1],
                            scale=-1.0,
                        )
                        if first:
                            nc.vector.tensor_mul(num[:, :], den[:, :], patch)
                            first = False
                        else:
                            t2 = tmp_pool.tile([P, W], FP32, tag="t2")
                            nc.vector.tensor_mul(t2[:, :], e_dst[:, :], patch)
                            nc.vector.tensor_add(num[:, :], num[:, :], t2[:, :])
                            nc.vector.tensor_add(den[:, :], den[:, :], e_dst[:, :])

                r = res_pool.tile([P, W], FP32, tag="r")
                nc.vector.reciprocal(r[:, :], den[:, :])
                nc.vector.tensor_mul(r[:, :], r[:, :], num[:, :])
                for c in range(CH_PER_TILE):
                    nc.sync.dma_start(
                        out=out[c0 + c, :, :],
                        in_=r[c * ROWS_PER_CORE:(c + 1) * ROWS_PER_CORE, :],
                    )
    nc.finalize()
    return nc


_NC_CACHE: dict = {}


def _get_nc(gw: np.ndarray) -> bass.Bass:
    key = gw.tobytes()
    if key not in _NC_CACHE:
        _NC_CACHE[key] = _build_nc(gw)
    return _NC_CACHE[key]


def run(x: np.ndarray, gw: np.ndarray, trace: bool = False):
    x = np.asarray(x, np.float32)
    gw = np.asarray(gw, np.float32)
    assert x.shape == (B, C, H, W) and gw.shape == (K, K)

    xp_full = np.pad(x, ((0, 0), (0, 0), (PAD, PAD), (PAD, PAD)), mode="edge")
    xp_full = xp_full.reshape(CH, H + 2 * PAD, SW)
    in_maps = []
    for k in range(NCORES):
        r0 = k * ROWS_PER_CORE
        strip = np.ascontiguousarray(xp_full[:, r0:r0 + SH, :])
        in_maps.append({"xp": strip})

    nc = _get_nc(gw)
    res = run_bass_kernel_spmd(nc, in_maps, list(range(NCORES)), trace=trace)

    full = np.empty((B, C, H, W), np.float32)
    for k in range(NCORES):
        o = res.results[k]["out"].reshape(B, C, ROWS_PER_CORE, W)
        full[:, :, k * ROWS_PER_CORE:(k + 1) * ROWS_PER_CORE, :] = o
    return full, res


def kernel(**inputs) -> np.ndarray:
    out, _ = run(inputs["x"], inputs["gw"])
    return out


# revision 16
# speedup vs baseline: 1.6169x; 1.6169x over previous
"""Bilateral filter (5x5, sigma_space = sigma_density = 1.1) on 8 TRN2 NeuronCores.

Contract: kernel(x, gw) takes FULL inputs
    x : [4, 3, 512, 512] float32
    gw: [5, 5] float32 (normalized spatial gaussian)
returns FULL output [4, 3, 512, 512] float32.

Sharding: pure data parallel over H. Core k processes output rows
[64k, 64k+64) of every (b, c) channel. The host pre-pads x with 2-pixel
edge replication on H and W and hands each core a [12, 68, 516] strip
(12 = 4*3 channels, 68 = 64 rows + 2+2 halo, 516 = 512 + 2+2 pad), so the
device kernel needs no boundary handling and no inter-core communication.

Device algorithm (v2, direct, fp16 compute):
  out = sum_t(w_t * p_t) / sum_t(w_t),  w_t = gw_t * exp(-(p_t - c)^2 / (2 s^2))
(the reference's per-window normalization of the density weights cancels in
the num/den ratio, so it is skipped).
fp16 throughout the tap loop: 16-bit dtypes give the DVE its 2x tensor_tensor
mode, and fp16's 11 mantissa bits keep the 25-term accumulations accurate.
Per tap: DVE subtract, square on ACT or DVE (split to balance engines),
ACT Exp(-u + ln(gw_t)) (folds the spatial weight in as the activation bias),
DVE multiply + two DVE accumulate-adds.
"""

import math

import numpy as np

import concourse.bass as bass
import concourse.bacc as bacc
import concourse.tile as tile
from concourse import mybir
from concourse.bass_utils import run_bass_kernel_spmd

# ---- problem constants (hardcoded per contract) ----
B, C, H, W = 4, 3, 512, 512
K = 5
PAD = 2
SIGMA = 0.3 * ((K - 1) * 0.5 - 1) + 0.8  # 1.1
NCORES = 8
CH = B * C                # 12 channels
ROWS_PER_CORE = H // NCORES   # 64
SH = ROWS_PER_CORE + 2 * PAD  # 68 rows per channel strip (with halo)
SW = W + 2 * PAD              # 516
P = 128                   # SBUF partitions
TILES = CH * ROWS_PER_CORE // P  # 6 tiles of 128 output rows per core
CH_PER_TILE = P // ROWS_PER_CORE  # 2 channels per tile

FP32 = mybir.dt.float32
FP16 = mybir.dt.float16

# fraction of taps whose square runs on DVE instead of ACT (engine balance)
DVE_SQUARE_EVERY = 8  # every 8th tap's square goes to DVE


def _build_nc(gw: np.ndarray) -> bass.Bass:
    inv2s2 = 1.0 / (2.0 * SIGMA * SIGMA)
    sq_scale = math.sqrt(inv2s2)  # Square(d * sq_scale) = d^2/(2 s^2)
    lgw = np.log(np.asarray(gw, np.float64)).astype(np.float32)  # [5,5]

    nc = bacc.Bacc(None)
    xp = nc.declare_dram_parameter("xp", [CH, SH, SW], FP16, isOutput=False)
    out = nc.declare_dram_parameter("out", [CH, ROWS_PER_CORE, W], FP32, isOutput=True)

    with tile.TileContext(nc) as tc:
        with (
            tc.tile_pool(name="singles", bufs=1) as singles,
            tc.tile_pool(name="shift", bufs=2) as shift_pool,
            tc.tile_pool(name="acc", bufs=4) as acc_pool,
            tc.tile_pool(name="tmp", bufs=4) as tmp_pool,
            tc.tile_pool(name="res", bufs=4) as res_pool,
        ):
            # per-tap activation bias ln(gw[i,j]) as [P,1] columns
            lgw_tile = singles.tile([P, K * K], FP32, tag="lgw")
            for idx in range(K * K):
                nc.vector.memset(
                    lgw_tile[:, idx:idx + 1], float(lgw[idx // K, idx % K])
                )
            for t in range(TILES):
                c0 = t * CH_PER_TILE
                # 5 row-shifted copies: S[p, i, :] = xp[ch(p), row(p)+i, :]
                # (plain 2D DMAs only — partition-split views mis-lower)
                S = shift_pool.tile([P, K, SW], FP16, tag="shift")
                for i in range(K):
                    for c in range(CH_PER_TILE):
                        nc.sync.dma_start(
                            out=S[c * ROWS_PER_CORE:(c + 1) * ROWS_PER_CORE,
                                  i, :],
                            in_=xp[c0 + c, i:i + ROWS_PER_CORE, :],
                        )
                center = S[:, PAD, PAD:PAD + W]

                num = acc_pool.tile([P, W], FP16, tag="num")
                den = acc_pool.tile([P, W], FP16, tag="den")

                first = True
                for i in range(K):
                    for j in range(K):
                        idx = i * K + j
                        patch = S[:, i, j:j + W]
                        d = tmp_pool.tile([P, W], FP16, tag="d")
                        nc.vector.tensor_sub(d[:, :], patch, center)
                        if idx % DVE_SQUARE_EVERY == DVE_SQUARE_EVERY - 1:
                            # u = d^2 (scale folded into the Exp below)
                            nc.vector.tensor_mul(d[:, :], d[:, :], d[:, :])
                            exp_scale = -inv2s2
                        else:
                            # u = d^2 / (2 s^2) on ACT
                            nc.scalar.activation(
                                d[:, :], d[:, :],
                                mybir.ActivationFunctionType.Square,
                                scale=sq_scale,
                            )
                            exp_scale = -1.0
                        # e = gw[i,j] * exp(-u)
                        e_dst = den if first else tmp_pool.tile([P, W], FP16, tag="e")
                        nc.scalar.activation(
                            e_dst[:, :], d[:, :],
                            mybir.ActivationFunctionType.Exp,
                            bias=lgw_tile[:, idx:idx + 1],
                            scale=exp_scale,
                        )
                        if first:
                            nc.vector.tensor_mul(num[:, :], den[:, :], patch)
                            first = False
                        else:
                            t2 = tmp_pool.tile([P, W], FP16, tag="t2")
                            nc.vector.tensor_mul(t2[:, :], e_dst[:, :], patch)
                            nc.vector.tensor_add(num[:, :], num[:, :], t2[:, :])
                            nc.vector.tensor_add(den[:, :], den[:, :], e_dst[:, :])

                denf = res_pool.tile([P, W], FP32, tag="denf")
                nc.vector.tensor_copy(denf[:, :], den[:, :])
                rec = res_pool.tile([P, W], FP32, tag="rec")
                nc.vector.reciprocal_approx_fast(rec[:, :], denf[:, :])
                numf = res_pool.tile([P, W], FP32, tag="numf")
                nc.vector.tensor_copy(numf[:, :], num[:, :])
                r = res_pool.tile([P, W], FP32, tag="r")
                nc.vector.tensor_mul(r[:, :], rec[:, :], numf[:, :])
                for c in range(CH_PER_TILE):
                    nc.sync.dma_start(
                        out=out[c0 + c, :, :],
                        in_=r[c * ROWS_PER_CORE:(c + 1) * ROWS_PER_CORE, :],
                    )
    nc.finalize()
    return nc


_NC_CACHE: dict = {}


def _get_nc(gw: np.ndarray) -> bass.Bass:
    key = gw.tobytes()
    if key not in _NC_CACHE:
        _NC_CACHE[key] = _build_nc(gw)
    return _NC_CACHE[key]


def run(x: np.ndarray, gw: np.ndarray, trace: bool = False):
    x = np.asarray(x, np.float32)
    gw = np.asarray(gw, np.float32)
    assert x.shape == (B, C, H, W) and gw.shape == (K, K)

    xp_full = np.pad(x, ((0, 0), (0, 0), (PAD, PAD), (PAD, PAD)), mode="edge")
    xp_full = xp_full.reshape(CH, H + 2 * PAD, SW).astype(np.float16)
    in_maps = []
    for k in range(NCORES):
        r0 = k * ROWS_PER_CORE
        strip = np.ascontiguousarray(xp_full[:, r0:r0 + SH, :])
        in_maps.append({"xp": strip})

    nc = _get_nc(gw)
    res = run_bass_kernel_spmd(nc, in_maps, list(range(NCORES)), trace=trace)

    full = np.empty((B, C, H, W), np.float32)
    for k in range(NCORES):
        o = res.results[k]["out"].reshape(B, C, ROWS_PER_CORE, W)
        full[:, :, k * ROWS_PER_CORE:(k + 1) * ROWS_PER_CORE, :] = o
    return full, res


def kernel(**inputs) -> np.ndarray:
    out, _ = run(inputs["x"], inputs["gw"])
    return out


# revision 18
# speedup vs baseline: 2.8673x; 1.7733x over previous
"""Bilateral filter (5x5, sigma_space = sigma_density = 1.1) on 8 TRN2 NeuronCores.

Contract: kernel(x, gw) takes FULL inputs
    x : [4, 3, 512, 512] float32
    gw: [5, 5] float32 (normalized spatial gaussian)
returns FULL output [4, 3, 512, 512] float32.

Sharding: pure data parallel over H. Core k owns output rows [64k, 64k+64)
of every (b, c) channel; the host hands it an edge-padded strip, so the
device kernel needs no boundary handling and no inter-core communication.

Device algorithm (v3): Taylor/separable-convolution reformulation.
With inv = 1/sigma^2 and f(u) = exp(-u^2 * inv / 2):
    exp(-(p-c)^2*inv/2) = f(p) * f(c) * exp(p*c*inv)
                        ~ f(p) * f(c) * sum_m (inv^m/m!) p^m c^m
so (f(c) cancels in the ratio, and the spatial gaussian gw = gwy x gwx
is separable):
    out = num/den,  den = sum_m CP_m . CONV2[G_m],  num = sum_m CP_m . CONV2[G_{m+1}]
where G_m = f(x) * x^m (a per-pixel field), CP_m = (inv^m/m!) c^m, and
CONV2 is the separable 5x5 spatial gaussian.

Layout: W(columns) on SBUF partitions. The W-direction conv becomes a
banded-matrix matmul on the (otherwise idle) TensorEngine; the H-direction
conv is 4 packed DVE ops (symmetric-kernel pairing) over all 6 fields at
once. All elementwise work in fp16 (DVE 2x mode), PSUM accumulation fp32.
"""

import math

import numpy as np

import concourse.bass as bass
import concourse.bacc as bacc
import concourse.tile as tile
from concourse import mybir
from concourse.bass_utils import run_bass_kernel_spmd

# ---- problem constants (hardcoded per contract) ----
B, C, H, W = 4, 3, 512, 512
K = 5
PAD = 2
SIGMA = 0.3 * ((K - 1) * 0.5 - 1) + 0.8  # 1.1
NCORES = 8
CH = B * C                    # 12 channels
RPC = H // NCORES             # 64 output rows per core
SR = RPC + 2 * PAD            # 68 input rows per channel strip
P = 128
NG = W // P                   # 4 column groups
FI = 816                      # free dim of input-row fields: CH*SR
FO = CH * RPC                 # 768 free dim of output-row tensors
M = 4                         # Taylor order: fields G_0..G_{M+1}
NF = M + 2                    # 6 fields

FP32 = mybir.dt.float32
FP16 = mybir.dt.float16
AL = mybir.AluOpType
AF = mybir.ActivationFunctionType


def _build_nc(gw: np.ndarray) -> bass.Bass:
    inv = 1.0 / (SIGMA * SIGMA)
    sq_scale = math.sqrt(inv / 2.0)   # Square(x*s) = x^2*inv/2
    gw64 = np.asarray(gw, np.float64)
    gwy = gw64.sum(axis=1)            # H-direction 1D kernel (shift i)
    ky0, ky1, ky2 = float(gwy[0]), float(gwy[1]), float(gwy[2])
    # H-conv with ky2 deferred (cancels in num/den):
    #   t1 = (p2 * ky0/ky1) + p1 ; S' = (t1 * ky1/ky2) + center

    nc = bacc.Bacc(None)
    xg = nc.declare_dram_parameter("xg", [NG, P, FI], FP16, isOutput=False)
    xe = nc.declare_dram_parameter("xe", [4, FI], FP16, isOutput=False)
    xc = nc.declare_dram_parameter("xc", [NG, P, FO], FP16, isOutput=False)
    b1d = nc.declare_dram_parameter("b1", [P, P], FP16, isOutput=False)
    b2d = nc.declare_dram_parameter("b2", [4, P], FP16, isOutput=False)
    out = nc.declare_dram_parameter("out", [NG, P, FO], FP32, isOutput=True)

    with tile.TileContext(nc) as tc:
        with (
            tc.tile_pool(name="const", bufs=1) as const_pool,
            tc.tile_pool(name="xin", bufs=1) as xin_pool,
            tc.tile_pool(name="fields", bufs=1) as fld_pool,
            tc.tile_pool(name="ws", bufs=2) as ws_pool,
            tc.tile_pool(name="ps", bufs=4, space="PSUM") as ps_pool,
            tc.tile_pool(name="work", bufs=1) as work_pool,
            tc.tile_pool(name="res", bufs=2) as res_pool,
        ):
            b1 = const_pool.tile([P, P], FP16, tag="b1")
            nc.sync.dma_start(out=b1[:, :], in_=b1d[:, :])
            b2 = const_pool.tile([4, P], FP16, tag="b2")
            nc.sync.dma_start(out=b2[:, :], in_=b2d[:, :])

            # --- inputs: all 4 column groups + 4-col tail stay resident ---
            X = []
            for g in range(NG):
                xt = xin_pool.tile([P, FI], FP16, tag=f"x{g}")
                nc.sync.dma_start(out=xt[:, :], in_=xg[g, :, :])
                X.append(xt)
            xet = xin_pool.tile([4, FI], FP16, tag="xe")
            nc.sync.dma_start(out=xet[:, :], in_=xe[:, :])

            def make_fields(dst, src, np_, tag):
                """dst[:, m*FI:(m+1)*FI] = f(src)*src^m for m=0..NF-1."""
                sq = work_pool.tile([np_, FI], FP16, tag=f"sq{tag}")
                nc.scalar.activation(sq[:, :], src[:, :], AF.Square,
                                     scale=sq_scale)
                nc.scalar.activation(dst[:, 0:FI], sq[:, :], AF.Exp,
                                     scale=-1.0)
                for m in range(1, NF):
                    nc.vector.tensor_mul(
                        dst[:, m * FI:(m + 1) * FI],
                        dst[:, (m - 1) * FI:m * FI], src[:, :])

            # fields for every group (kept resident: edge mm needs neighbors)
            G = []
            for g in range(NG):
                gt = fld_pool.tile([P, NF * FI], FP16, tag=f"g{g}")
                make_fields(gt, X[g], P, f"g{g}")
                G.append(gt)
            ge = fld_pool.tile([4, NF * FI], FP16, tag="ge")
            make_fields(ge, xet, 4, "ge")

            HC = 408  # PSUM chunk (<=512 fp32 per bank)
            for g in range(NG):
                # --- W-conv on TensorE: WS_m = B^T @ G_m (banded 5-tap) ---
                ws = ws_pool.tile([P, NF * FI], FP16, tag="ws")
                nbr = G[g + 1] if g + 1 < NG else ge
                for m in range(NF):
                    for o in (0, HC):
                        pt = ps_pool.tile([P, HC], FP32, tag="pt")
                        sl = slice(m * FI + o, m * FI + o + HC)
                        nc.tensor.matmul(pt[:, :], b1[:, :], G[g][:, sl],
                                         start=True, stop=False)
                        nc.tensor.matmul(pt[:, :], b2[:, :], nbr[0:4, sl],
                                         start=False, stop=True)
                        nc.scalar.activation(ws[:, sl], pt[:, :], AF.Copy)

                # --- H-conv (packed over 6 fields x 12 channels x 64 rows) ---
                def hview(t, o, fsz, n=NF):
                    # [fields][channels][64 rows] at row-offset o
                    return bass.AP(tensor=t[:, :].tensor, offset=t[:, :].offset + o,
                                   ap=[list(t[:, :].ap[0]), [fsz, n],
                                       [SR, CH], [1, RPC]])

                p2 = work_pool.tile([P, NF, CH, RPC], FP16, tag="p2")
                nc.vector.tensor_add(p2[:, :, :, :], hview(ws, 0, FI),
                                     hview(ws, 4, FI))
                p1 = work_pool.tile([P, NF, CH, RPC], FP16, tag="p1")
                nc.vector.tensor_add(p1[:, :, :, :], hview(ws, 1, FI),
                                     hview(ws, 3, FI))
                t1 = work_pool.tile([P, NF, CH, RPC], FP16, tag="t1")
                nc.vector.scalar_tensor_tensor(
                    t1[:, :, :, :], p2[:, :, :, :], ky0 / ky1,
                    p1[:, :, :, :], op0=AL.mult, op1=AL.add)
                S = work_pool.tile([P, NF * FO], FP16, tag="S")
                Sv = S[:, :].rearrange("p (f c r) -> p f c r", f=NF, c=CH)
                nc.vector.scalar_tensor_tensor(
                    Sv, t1[:, :, :, :], ky1 / ky2,
                    hview(ws, 2, FI), op0=AL.mult, op1=AL.add)

                def sf(m):
                    return S[:, m * FO:(m + 1) * FO]

                # --- CP_m = (inv^m/m!) c^m ---
                c = res_pool.tile([P, FO], FP16, tag="c")
                nc.sync.dma_start(out=c[:, :], in_=xc[g, :, :])
                CP = res_pool.tile([P, M, FO], FP16, tag="cp")
                nc.vector.tensor_scalar_mul(CP[:, 0, :], c[:, :], inv)
                for m in range(2, M + 1):
                    nc.vector.scalar_tensor_tensor(
                        CP[:, m - 1, :], CP[:, m - 2, :], inv / m,
                        c[:, :], op0=AL.mult, op1=AL.mult)

                # --- num/den series ---
                def series(base_field):
                    acc = res_pool.tile([P, FO], FP16, tag=f"acc{base_field}")
                    t = res_pool.tile([P, FO], FP16, tag=f"t{base_field}")
                    nc.vector.tensor_mul(t[:, :], CP[:, 0, :], sf(base_field + 1))
                    nc.vector.tensor_add(acc[:, :], sf(base_field), t[:, :])
                    for m in range(2, M + 1):
                        nc.vector.tensor_mul(t[:, :], CP[:, m - 1, :],
                                             sf(base_field + m))
                        nc.vector.tensor_add(acc[:, :], acc[:, :], t[:, :])
                    return acc

                den = series(0)
                num = series(1)

                # --- out = num/den (fp32) ---
                denf = res_pool.tile([P, FO], FP32, tag="denf")
                nc.scalar.activation(denf[:, :], den[:, :], AF.Copy)
                numf = res_pool.tile([P, FO], FP32, tag="numf")
                nc.scalar.activation(numf[:, :], num[:, :], AF.Copy)
                rec = res_pool.tile([P, FO], FP32, tag="rec")
                nc.vector.reciprocal_approx_fast(rec[:, :], denf[:, :])
                r = res_pool.tile([P, FO], FP32, tag="r")
                nc.vector.tensor_mul(r[:, :], rec[:, :], numf[:, :])
                nc.sync.dma_start(out=out[g, :, :], in_=r[:, :])
    nc.finalize()
    return nc


_NC_CACHE: dict = {}


def _get_nc(gw: np.ndarray) -> bass.Bass:
    key = gw.tobytes()
    if key not in _NC_CACHE:
        _NC_CACHE[key] = _build_nc(gw)
    return _NC_CACHE[key]


def _host_prep(x: np.ndarray, gw: np.ndarray):
    """Shard + relayout on host. Returns in_maps for the 8 cores."""
    xp = np.pad(x, ((0, 0), (0, 0), (PAD, PAD), (PAD, PAD)), mode="edge")
    xp = xp.reshape(CH, H + 2 * PAD, W + 2 * PAD)          # [12, 516, 516]
    xp16 = xp.astype(np.float16)

    gw64 = np.asarray(gw, np.float64)
    gwx = gw64.sum(axis=0)   # W-direction 1D kernel (shift j)
    b1 = np.zeros((P, P), np.float16)
    b2 = np.zeros((4, P), np.float16)
    for mcol in range(P):
        for j in range(K):
            k = mcol + j
            if k < P:
                b1[k, mcol] = gwx[j]
            else:
                b2[k - P, mcol] = gwx[j]

    in_maps = []
    for core in range(NCORES):
        r0 = core * RPC
        strip = xp16[:, r0:r0 + SR, :]                     # [12, 68, 516]
        swt = strip.transpose(2, 0, 1)                     # [516, 12, 68]
        xg = np.ascontiguousarray(
            swt[:W].reshape(NG, P, FI))                    # [4, 128, 816]
        xe = np.ascontiguousarray(swt[W:].reshape(4, FI))  # [4, 816]
        ctr = strip[:, PAD:PAD + RPC, PAD:PAD + W]         # [12, 64, 512]
        xc = np.ascontiguousarray(
            ctr.transpose(2, 0, 1).reshape(NG, P, FO))     # [4, 128, 768]
        in_maps.append({"xg": xg, "xe": xe, "xc": xc, "b1": b1, "b2": b2})
    return in_maps


def run(x: np.ndarray, gw: np.ndarray, trace: bool = False):
    x = np.asarray(x, np.float32)
    gw = np.asarray(gw, np.float32)
    assert x.shape == (B, C, H, W) and gw.shape == (K, K)

    in_maps = _host_prep(x, gw)
    nc = _get_nc(gw)
    res = run_bass_kernel_spmd(nc, in_maps, list(range(NCORES)), trace=trace)

    full = np.empty((B, C, H, W), np.float32)
    for core in range(NCORES):
        o = res.results[core]["out"].reshape(W, CH, RPC)   # [512, 12, 64]
        o = o.transpose(1, 2, 0).reshape(B, C, RPC, W)
        full[:, :, core * RPC:(core + 1) * RPC, :] = o
    return full, res


def kernel(**inputs) -> np.ndarray:
    out, _ = run(inputs["x"], inputs["gw"])
    return out


# revision 19
# speedup vs baseline: 3.6587x; 1.2760x over previous
"""Bilateral filter (5x5, sigma_space = sigma_density = 1.1) on 8 TRN2 NeuronCores.

Contract: kernel(x, gw) takes FULL inputs
    x : [4, 3, 512, 512] float32
    gw: [5, 5] float32 (normalized spatial gaussian)
returns FULL output [4, 3, 512, 512] float32.

Sharding: pure data parallel over H. Core k owns output rows [64k, 64k+64)
of every (b, c) channel; the host hands it an edge-padded strip, so the
device kernel needs no boundary handling and no inter-core communication.

Device algorithm (v4): Taylor/separable-convolution reformulation.
With inv = 1/sigma^2 and f(u) = exp(-u^2 * inv / 2):
    exp(-(p-c)^2*inv/2) = f(p) * f(c) * exp(p*c*inv)
                        ~ f(p) * f(c) * sum_{m<=M} (inv^m/m!) p^m c^m
so (f(c) cancels in the num/den ratio, and gw = gwy x gwx is separable):
    out = num/den,  den = sum_m CP_m . CONV2[G_m],  num = sum_m CP_m . CONV2[G_{m+1}]
where G_m = f(x) * x^m (a per-pixel field), CP_m = (inv^m/m!) c^m, and
CONV2 is the separable 5x5 spatial gaussian. M=3 -> 5 fields, truncation
error ~6e-4 relative.

Layout: W(columns) on SBUF partitions; free dim is [row][channel] so every
H-direction row shift lands 4B-aligned (keeps the DVE fp16 2x/4x modes).
The W-direction conv is a banded-matrix matmul on the otherwise idle
TensorEngine (fp32 PSUM accumulation); the H-direction conv is 6 packed
DVE ops (symmetric-kernel pairing, scale steps on 4x tensor_scalar) over
all 5 fields at once. All elementwise work in fp16.
"""

import math

import numpy as np

import concourse.bass as bass
import concourse.bacc as bacc
import concourse.tile as tile
from concourse import mybir
from concourse.bass_utils import run_bass_kernel_spmd

# ---- problem constants (hardcoded per contract) ----
B, C, H, W = 4, 3, 512, 512
K = 5
PAD = 2
SIGMA = 0.3 * ((K - 1) * 0.5 - 1) + 0.8  # 1.1
NCORES = 8
CH = B * C                    # 12 channels
RPC = H // NCORES             # 64 output rows per core
SR = RPC + 2 * PAD            # 68 input rows per channel strip
P = 128
NG = W // P                   # 4 column groups
FI = SR * CH                  # 816 free elems of input-row fields [row][ch]
FO = RPC * CH                 # 768 free elems of output-row tensors [row][ch]
M = 3                         # Taylor order: fields G_0..G_{M+1}
NF = M + 2                    # 5 fields

FP32 = mybir.dt.float32
FP16 = mybir.dt.float16
AL = mybir.AluOpType
AF = mybir.ActivationFunctionType


def _build_nc(gw: np.ndarray) -> bass.Bass:
    inv = 1.0 / (SIGMA * SIGMA)
    sq_scale = math.sqrt(inv / 2.0)   # Square(x*s) = x^2*inv/2
    gw64 = np.asarray(gw, np.float64)
    gwy = gw64.sum(axis=1)            # H-direction 1D kernel (shift i)
    ky0, ky1, ky2 = float(gwy[0]), float(gwy[1]), float(gwy[2])
    # H-conv with ky2 deferred (uniform scale cancels in num/den):
    #   S' = ((p2 * ky0/ky1) + p1) * ky1/ky2 + center

    nc = bacc.Bacc(None)
    xg = nc.declare_dram_parameter("xg", [NG, P, FI], FP16, isOutput=False)
    xe = nc.declare_dram_parameter("xe", [4, FI], FP16, isOutput=False)
    xc = nc.declare_dram_parameter("xc", [NG, P, FO], FP16, isOutput=False)
    b1d = nc.declare_dram_parameter("b1", [P, P], FP16, isOutput=False)
    b2d = nc.declare_dram_parameter("b2", [4, P], FP16, isOutput=False)
    out = nc.declare_dram_parameter("out", [NG, P, FO], FP32, isOutput=True)

    with tile.TileContext(nc) as tc:
        with (
            tc.tile_pool(name="const", bufs=1) as const_pool,
            tc.tile_pool(name="xin", bufs=1) as xin_pool,
            tc.tile_pool(name="fields", bufs=1) as fld_pool,
            tc.tile_pool(name="ws", bufs=2) as ws_pool,
            tc.tile_pool(name="ps", bufs=4, space="PSUM") as ps_pool,
            tc.tile_pool(name="work", bufs=1) as work_pool,
            tc.tile_pool(name="res", bufs=2) as res_pool,
        ):
            b1 = const_pool.tile([P, P], FP16, tag="b1")
            nc.sync.dma_start(out=b1[:, :], in_=b1d[:, :])
            b2 = const_pool.tile([4, P], FP16, tag="b2")
            nc.sync.dma_start(out=b2[:, :], in_=b2d[:, :])

            # --- inputs: all 4 column groups + 4-col tail stay resident ---
            X = []
            for g in range(NG):
                xt = xin_pool.tile([P, FI], FP16, tag=f"x{g}")
                nc.sync.dma_start(out=xt[:, :], in_=xg[g, :, :])
                X.append(xt)
            xet = xin_pool.tile([4, FI], FP16, tag="xe")
            nc.sync.dma_start(out=xet[:, :], in_=xe[:, :])

            def make_fields(dst, src, np_, tag):
                """dst[:, m*FI:(m+1)*FI] = f(src)*src^m for m=0..NF-1."""
                sq = work_pool.tile([np_, FI], FP16, tag=f"sq{tag}")
                nc.scalar.activation(sq[:, :], src[:, :], AF.Square,
                                     scale=sq_scale)
                nc.scalar.activation(dst[:, 0:FI], sq[:, :], AF.Exp,
                                     scale=-1.0)
                for m in range(1, NF):
                    nc.vector.tensor_mul(
                        dst[:, m * FI:(m + 1) * FI],
                        dst[:, (m - 1) * FI:m * FI], src[:, :])

            # fields for every group (kept resident: edge mm needs neighbors)
            G = []
            for g in range(NG):
                gt = fld_pool.tile([P, NF * FI], FP16, tag=f"g{g}")
                make_fields(gt, X[g], P, f"g{g}")
                G.append(gt)
            ge = fld_pool.tile([4, NF * FI], FP16, tag="ge")
            make_fields(ge, xet, 4, "ge")

            HC = 408  # PSUM chunk (<=512 fp32 per bank)
            for g in range(NG):
                # --- W-conv on TensorE: WS_m = B^T @ G_m (banded 5-tap) ---
                ws = ws_pool.tile([P, NF * FI], FP16, tag="ws")
                nbr = G[g + 1] if g + 1 < NG else ge
                for m in range(NF):
                    for o in (0, HC):
                        pt = ps_pool.tile([P, HC], FP32, tag="pt")
                        sl = slice(m * FI + o, m * FI + o + HC)
                        nc.tensor.matmul(pt[:, :], b1[:, :], G[g][:, sl],
                                         start=True, stop=False)
                        nc.tensor.matmul(pt[:, :], b2[:, :], nbr[0:4, sl],
                                         start=False, stop=True)
                        nc.scalar.activation(ws[:, sl], pt[:, :], AF.Copy)

                # --- H-conv, packed over 5 fields x 64 rows x 12 channels ---
                def hview(t, o):
                    # fields x rows(out) x channels at row-offset o
                    base = t[:, :]
                    return bass.AP(tensor=base.tensor,
                                   offset=base.offset + o * CH,
                                   ap=[list(base.ap[0]), [FI, NF],
                                       [CH, RPC], [1, CH]])

                p2 = work_pool.tile([P, NF, RPC, CH], FP16, tag="p2")
                nc.vector.tensor_add(p2[:, :, :, :], hview(ws, 0),
                                     hview(ws, 4))
                p1 = work_pool.tile([P, NF, RPC, CH], FP16, tag="p1")
                nc.vector.tensor_add(p1[:, :, :, :], hview(ws, 1),
                                     hview(ws, 3))
                nc.vector.tensor_scalar_mul(p2[:, :, :, :], p2[:, :, :, :],
                                            ky0 / ky1)
                nc.vector.tensor_add(p1[:, :, :, :], p1[:, :, :, :],
                                     p2[:, :, :, :])
                nc.vector.tensor_scalar_mul(p1[:, :, :, :], p1[:, :, :, :],
                                            ky1 / ky2)
                S = work_pool.tile([P, NF * FO], FP16, tag="S")
                Sv = S[:, :].rearrange("p (f r c) -> p f r c", f=NF, r=RPC)
                nc.vector.tensor_add(Sv, p1[:, :, :, :], hview(ws, 2))

                def sf(m):
                    return S[:, m * FO:(m + 1) * FO]

                # --- CP_m = (inv^m/m!) c^m ---
                c = res_pool.tile([P, FO], FP16, tag="c")
                nc.sync.dma_start(out=c[:, :], in_=xc[g, :, :])
                CP = res_pool.tile([P, M, FO], FP16, tag="cp")
                nc.vector.tensor_scalar_mul(CP[:, 0, :], c[:, :], inv)
                for m in range(2, M + 1):
                    nc.vector.tensor_scalar_mul(CP[:, m - 1, :],
                                                CP[:, m - 2, :], inv / m)
                    nc.vector.tensor_mul(CP[:, m - 1, :], CP[:, m - 1, :],
                                         c[:, :])

                # --- num/den series ---
                def series(base_field):
                    acc = res_pool.tile([P, FO], FP16, tag=f"acc{base_field}")
                    t = res_pool.tile([P, FO], FP16, tag=f"t{base_field}")
                    nc.vector.tensor_mul(t[:, :], CP[:, 0, :], sf(base_field + 1))
                    nc.vector.tensor_add(acc[:, :], sf(base_field), t[:, :])
                    for m in range(2, M + 1):
                        nc.vector.tensor_mul(t[:, :], CP[:, m - 1, :],
                                             sf(base_field + m))
                        nc.vector.tensor_add(acc[:, :], acc[:, :], t[:, :])
                    return acc

                den = series(0)
                num = series(1)

                # --- out = num/den (fp32) ---
                denf = res_pool.tile([P, FO], FP32, tag="denf")
                nc.scalar.activation(denf[:, :], den[:, :], AF.Copy)
                numf = res_pool.tile([P, FO], FP32, tag="numf")
                nc.scalar.activation(numf[:, :], num[:, :], AF.Copy)
                rec = res_pool.tile([P, FO], FP32, tag="rec")
                nc.vector.reciprocal_approx_fast(rec[:, :], denf[:, :])
                r = res_pool.tile([P, FO], FP32, tag="r")
                nc.vector.tensor_mul(r[:, :], rec[:, :], numf[:, :])
                nc.sync.dma_start(out=out[g, :, :], in_=r[:, :])
    nc.finalize()
    return nc


_NC_CACHE: dict = {}


def _get_nc(gw: np.ndarray) -> bass.Bass:
    key = gw.tobytes()
    if key not in _NC_CACHE:
        _NC_CACHE[key] = _build_nc(gw)
    return _NC_CACHE[key]


def _host_prep(x: np.ndarray, gw: np.ndarray):
    """Shard + relayout on host. Returns in_maps for the 8 cores."""
    xp = np.pad(x, ((0, 0), (0, 0), (PAD, PAD), (PAD, PAD)), mode="edge")
    xp = xp.reshape(CH, H + 2 * PAD, W + 2 * PAD)          # [12, 516, 516]
    xp16 = xp.astype(np.float16)

    gw64 = np.asarray(gw, np.float64)
    gwx = gw64.sum(axis=0)   # W-direction 1D kernel (shift j)
    b1 = np.zeros((P, P), np.float16)
    b2 = np.zeros((4, P), np.float16)
    for mcol in range(P):
        for j in range(K):
            k = mcol + j
            if k < P:
                b1[k, mcol] = gwx[j]
            else:
                b2[k - P, mcol] = gwx[j]

    in_maps = []
    for core in range(NCORES):
        r0 = core * RPC
        strip = xp16[:, r0:r0 + SR, :]                     # [12, 68, 516]
        swt = strip.transpose(2, 1, 0)                     # [516, 68, 12]
        xgv = np.ascontiguousarray(
            swt[:W].reshape(NG, P, FI))                    # [4, 128, 816]
        xev = np.ascontiguousarray(swt[W:].reshape(4, FI))  # [4, 816]
        ctr = strip[:, PAD:PAD + RPC, PAD:PAD + W]         # [12, 64, 512]
        xcv = np.ascontiguousarray(
            ctr.transpose(2, 1, 0).reshape(NG, P, FO))     # [4, 128, 768]
        in_maps.append({"xg": xgv, "xe": xev, "xc": xcv, "b1": b1, "b2": b2})
    return in_maps


def run(x: np.ndarray, gw: np.ndarray, trace: bool = False):
    x = np.asarray(x, np.float32)
    gw = np.asarray(gw, np.float32)
    assert x.shape == (B, C, H, W) and gw.shape == (K, K)

    in_maps = _host_prep(x, gw)
    nc = _get_nc(gw)
    res = run_bass_kernel_spmd(nc, in_maps, list(range(NCORES)), trace=trace)

    full = np.empty((B, C, H, W), np.float32)
    for core in range(NCORES):
        o = res.results[core]["out"].reshape(W, RPC, CH)   # [512, 64, 12]
        o = o.transpose(2, 1, 0).reshape(B, C, RPC, W)
        full[:, :, core * RPC:(core + 1) * RPC, :] = o
    return full, res


def kernel(**inputs) -> np.ndarray:
    out, _ = run(inputs["x"], inputs["gw"])
    return out
